# revision 37
# baseline (speedup 1.0000x reference)
"""Multi-head attention (B=4, N=2048, C=1024, H=16) on 8 trn2 NeuronCores.

Sharding: data-parallel over batch (4) x tensor-parallel over heads (2).
Core c handles batch c//2, heads [8*(c%2), 8*(c%2)+8). Each core computes a
partial output projection (contraction over its 512 channels); the host sums
core pairs and adds the projection bias.

Device-side math per core (n=2048 tokens, cp=512 channels, 8 heads, hd=64):
  qT/kT = (w @ x^T) in transposed layout [c', n]; v in natural layout [n, c']
  augmented with a ones column per head (gives the softmax denominator for
  free as row 64 of the attn@V matmul). Scores are computed transposed
  [k, q]; each PSUM score tile packs an even/odd head pair (cols 0:512 /
  512:1024 on PE row groups 0 / 64). exp on ScalarE (no max subtraction;
  logits are bounded), mask applied as one broadcast bf16 multiply on
  VectorE, attn@V + denominator on TensorE, normalization via a rank-1
  ones x dinv broadcast matmul, then the output projection.

  Dtypes: x/w/v/wp/attn-probs/aoT/out in bf16 (halves DMA + enables fast
  weight load); scores q/k in fp32r (full PE rate at free dim >= 256) with
  fp32 PSUM accumulation everywhere. Phase-1 x is DMA'd in token chunks
  interleaved with the weights so the first matmuls start after ~2MB.
"""

import os
import sys

for p in ("/opt/trn_rl_repo", "/root/.axon_site/_ro/trn_rl_repo"):
    if os.path.isdir(p) and p not in sys.path:
        sys.path.insert(0, p)

import ml_dtypes
import numpy as np

import concourse.bacc as bacc
import concourse.tile as tile
from concourse import mybir
from concourse.bass_utils import run_bass_kernel_spmd

FP = mybir.dt.float32
FR = mybir.dt.float32r
BF = mybir.dt.bfloat16
EXP = mybir.ActivationFunctionType.Exp

DIM = 1024
NUM_HEADS = 16
HEAD_DIM = 64
SCALE = HEAD_DIM ** -0.5
B, N = 4, 2048
NCORES = 8


def build_attention(n=N, c=DIM, cp=DIM // 2, hd=HEAD_DIM, scale=SCALE):
    """Emit the per-core program. All cores run the same code (SPMD)."""
    hpc = cp // hd          # heads on this core
    CB = c // 128           # contraction blocks for QKV
    MB = cp // 128          # c' blocks (q/k transposed layout)
    NB = n // 128           # token blocks
    QC = n // 512
    QW = min(1024, n)       # phase-2 q chunk width
    QH = n // QW            # q chunks (phase-2 outer loop)
    hd1 = hd + 1            # v augmented with a ones column -> denominator

    nc = bacc.Bacc("TRN2", target_bir_lowering=False, debug=False)

    xT = nc.dram_tensor("xT", [c, n], BF, kind="ExternalInput").ap()
    wqT = nc.dram_tensor("wqT", [c, cp], BF, kind="ExternalInput").ap()
    wkT = nc.dram_tensor("wkT", [c, cp], BF, kind="ExternalInput").ap()
    wvT = nc.dram_tensor("wvT", [c, cp], BF, kind="ExternalInput").ap()
    wpT = nc.dram_tensor("wpT", [cp, c], BF, kind="ExternalInput").ap()
    maskT = nc.dram_tensor("maskT", [n, n], BF, kind="ExternalInput").ap()
    out = nc.dram_tensor("out", [n, c], BF, kind="ExternalOutput").ap()

    with tile.TileContext(nc) as tc:
        with (
            tc.tile_pool(name="persist", bufs=1) as pers,
            tc.tile_pool(name="d_pool", bufs=2) as dpool,
        ):
            qT_sb = pers.tile([128, MB, n], FR, tag="qT")
            d_sb = dpool.tile([hpc, n], FP, tag="dsum", bufs=1)
            dinv_sb = dpool.tile([hpc, n], FP, tag="dinv", bufs=1)
            kT_sb = pers.tile([128, MB, n], FR, tag="kT")
            vaug_sb = pers.tile([128, NB, hpc * hd1], BF, tag="vaug")

            # ---------------- Phase 1: QKV projections ----------------
            # DMA is issued token-chunk-interleaved so the first k/q matmuls
            # start after ~4MB (wk + x chunk 0) instead of the full 14MB.
            with (
                tc.tile_pool(name="xt", bufs=2) as xpool,
                tc.tile_pool(name="w", bufs=3) as wpool,
                tc.tile_pool(name="ps_qkv", bufs=4, space="PSUM") as pq,
            ):
                x3 = xT.rearrange("(cb p) n -> p cb n", p=128)
                w_aps = {"q": wqT, "k": wkT, "v": wvT}
                w_sb = {
                    wn: wpool.tile([128, CB, cp], BF, tag="w", name=f"w_{wn}")
                    for wn in ("q", "k", "v")
                }
                x_tiles = {}

                def dma_x_chunk(qc):
                    xc = xpool.tile([128, CB, 512], BF, tag="xc", name="xc")
                    for cb in range(CB):
                        nc.sync.dma_start(
                            xc[:, cb, :],
                            x3[:, cb, qc * 512:(qc + 1) * 512],
                        )
                    x_tiles[qc] = xc

                nc.sync.dma_start(
                    w_sb["k"], w_aps["k"].rearrange("(cb p) m -> p cb m", p=128)
                )
                dma_x_chunk(0)
                nc.sync.dma_start(
                    w_sb["q"], w_aps["q"].rearrange("(cb p) m -> p cb m", p=128)
                )
                dma_x_chunk(1)
                nc.sync.dma_start(
                    w_sb["v"], w_aps["v"].rearrange("(cb p) m -> p cb m", p=128)
                )

                # per token chunk: kT, qT blocks then v blocks for that chunk
                for qc in range(QC):
                    xc = x_tiles.pop(qc)
                    for wn, dst in (("k", kT_sb), ("q", qT_sb)):
                        for mb in range(MB):
                            pt = pq.tile([128, 512], FP, tag="psqkv")
                            for cb in range(CB):
                                nc.tensor.matmul(
                                    pt,
                                    lhsT=w_sb[wn][:, cb, mb * 128:(mb + 1) * 128],
                                    rhs=xc[:, cb, :],
                                    start=(cb == 0),
                                    stop=(cb == CB - 1),
                                )
                            nc.vector.tensor_copy(
                                dst[:, mb, qc * 512:(qc + 1) * 512], pt
                            )
                    # v natural layout [n block, c']; lhsT = xT token block
                    for nbi in range(4):
                        nb = 4 * qc + nbi
                        pt = pq.tile([128, cp], FP, tag="psqkv")
                        for cb in range(CB):
                            nc.tensor.matmul(
                                pt,
                                lhsT=xc[:, cb, nbi * 128:(nbi + 1) * 128],
                                rhs=w_sb["v"][:, cb, :],
                                start=(cb == 0),
                                stop=(cb == CB - 1),
                            )
                        dst3 = vaug_sb[:, nb, :].rearrange(
                            "p (h e) -> p h e", e=hd1
                        )
                        nc.vector.tensor_copy(
                            dst3[:, :, 0:hd],
                            pt.rearrange("p (h e) -> p h e", e=hd),
                        )
                        nc.vector.memset(dst3[:, :, hd:hd1], 1.0)
                    if qc + 2 <= QC - 1:
                        dma_x_chunk(qc + 2)

            # ---------------- Phase 2: scores / softmax / attn@V ------------
            with (
                tc.tile_pool(name="aoT", bufs=1) as aop,
                tc.tile_pool(name="wp", bufs=1) as wppool,
            ):
                aoT_sb = aop.tile([128, MB, n], BF, tag="aoT")
                wp_sb = wppool.tile([128, MB, c], BF, tag="wp")
                with (
                    tc.tile_pool(name="mask", bufs=1) as mpool,
                    tc.tile_pool(name="ps_sc", bufs=2, space="PSUM") as psc,
                    tc.tile_pool(name="ps_ao", bufs=2, space="PSUM") as pao,
                    tc.tile_pool(name="s_exp", bufs=5) as sep,
                    tc.tile_pool(name="s_m", bufs=5) as smp,
                ):
                    for qh in range(QH):
                        qo = qh * QW
                        mk = mpool.tile([128, NB, QW], BF, tag="maskT")
                        for kb in range(NB):
                            nc.sync.dma_start(
                                mk[:, kb, :],
                                maskT.rearrange("(kb p) q -> p kb q", p=128)[
                                    :, kb, qo:qo + QW
                                ],
                            )
                        if qh == 0:
                            # preload the projection weights behind the first
                            # mask chunk so the tail never waits on this DMA
                            nc.sync.dma_start(
                                wp_sb,
                                wpT.rearrange("(mb p) co -> p mb co", p=128),
                            )
                        # software-pipelined over tile-units (pb, kb, qs):
                        # one PSUM score tile holds a 512-wide q slice for an
                        # even/odd head pair (h0 -> cols 0:512 on PE row group
                        # 0, h1 -> cols 512:1024 on row group 64, different
                        # PSUM banks) so the two K=64 score matmuls stream
                        # CONCURRENTLY through the PE array (row tiling).
                        units = [
                            (pb, kb, qs)
                            for pb in range(hpc // 2)
                            for kb in range(NB)
                            for qs in range(QW // 512)
                        ]
                        LOOK = 2
                        sc_map = {}
                        ao_map = {}
                        sm_map = {}
                        for idx in range(len(units) + LOOK):
                            if idx < len(units):
                                pb, kb, qs = units[idx]
                                hb = pb
                                sc_t = psc.tile([128, QW], FP, tag="sc")
                                sc_map[idx] = sc_t
                                qlo = qo + qs * 512
                                for d in (0, 1):
                                    po = d * hd
                                    nc.tensor.matmul(
                                        sc_t[:, d * 512:(d + 1) * 512],
                                        lhsT=kT_sb[po:po + hd, hb, kb * 128:(kb + 1) * 128],
                                        rhs=qT_sb[po:po + hd, hb, qlo:qlo + 512],
                                        start=True,
                                        stop=True,
                                    )
                            j = idx - LOOK
                            if j < 0:
                                continue
                            pb, kb, qs = units[j]
                            hb = pb
                            sc = sc_map.pop(j)
                            se = sep.tile([128, QW], BF, tag="se")
                            nc.scalar.activation(se, sc, EXP, scale=scale)
                            if qs == 0:
                                # per (pair, kb) masked-prob tile laid out
                                # [head d, q 0:QW] so each head's attn@V can
                                # stream a single contiguous 1024-wide rhs
                                sm_map.clear()
                                sm_map[0] = smp.tile([128, 2, QW], BF, tag="sm", name="sm2")
                            sm2 = sm_map[0]
                            # one DVE mul: the 512-wide mask slice is
                            # broadcast across the two head halves
                            mslice = mk[:, kb, qs * 512:(qs + 1) * 512]
                            nc.vector.tensor_mul(
                                sm2[:, :, qs * 512:(qs + 1) * 512],
                                se.rearrange("p (d q) -> p d q", d=2),
                                mslice.rearrange(
                                    "p (o q) -> p o q", o=1
                                ).broadcast_to([128, 2, 512]),
                            )
                            if kb == 0 and qs == 0:
                                ao_map[pb] = (
                                    pao.tile([hd1, QW], FP, tag="ao", name="ao0"),
                                    pao.tile([hd1, QW], FP, tag="ao", name="ao1"),
                                )
                            if qs == QW // 512 - 1:
                                # attn@V for both heads, both q slices; the two
                                # matmuls sharing one lhsT stay adjacent
                                for d in (0, 1):
                                    h = 2 * pb + d
                                    for qq in range(QW // 512):
                                        nc.tensor.matmul(
                                            ao_map[pb][d][:, qq * 512:(qq + 1) * 512],
                                            lhsT=vaug_sb[:, kb, h * hd1:(h + 1) * hd1],
                                            rhs=sm2[:, d, qq * 512:(qq + 1) * 512],
                                            start=(kb == 0),
                                            stop=(kb == NB - 1),
                                        )
                            if kb == NB - 1 and qs == QW // 512 - 1:
                                for d in (0, 1):
                                    h = 2 * pb + d
                                    po = d * hd
                                    ao = ao_map[pb][d]
                                    # on ScalarE: halves the boundary DVE
                                    # chain that head-of-line blocks the next
                                    # pair's mask multiplies
                                    nc.scalar.copy(
                                        aoT_sb[po:po + hd, hb, qo:qo + QW],
                                        ao[0:hd, :],
                                    )
                                    # D row: PSUM partition 64 -> partition-0
                                    # SBUF tile (aligned start partitions),
                                    # then DMA into row h of the batch tile.
                                    dtmp = dpool.tile([1, QW], FP, tag="dtmp")
                                    nc.vector.tensor_copy(dtmp, ao[hd:hd1, :])
                                    nc.scalar.dma_start(
                                        d_sb[h:h + 1, qo:qo + QW], dtmp
                                    )
                                if pb == hpc // 2 - 1:
                                    # all 8 denominator rows for this q window
                                    # are in: reciprocal now, off the tail's
                                    # critical path
                                    nc.vector.reciprocal_approx_fast(
                                        dinv_sb[:, qo:qo + QW],
                                        d_sb[:, qo:qo + QW],
                                    )
                                del ao_map[pb]

                # ---- normalization + output projection. The broadcast
                # matmuls for heads 0..5 have no pending inputs (their 1/D
                # rows were computed during phase 2), so they open the tail
                # and keep the PE clock warm while the last pair's
                # denominator chain drains; then the projection streams. ----
                with (
                    tc.tile_pool(name="dinv", bufs=4) as dip,
                    tc.tile_pool(name="ps_bc", bufs=2, space="PSUM") as pbc,
                    tc.tile_pool(name="ps_o", bufs=4, space="PSUM") as pso,
                    tc.tile_pool(name="osb", bufs=3) as osp,
                ):
                    ones_raw = dip.tile([1, hd], FP, tag="ones_raw", bufs=1)
                    nc.vector.memset(ones_raw, 1.0)
                    ones_sb = dip.tile([1, hd], FR, tag="ones", bufs=1)
                    nc.vector.tensor_copy(ones_sb, ones_raw)
                    norm_order = [
                        (qc, h) for qc in range(QC) for h in range(hpc - 2)
                    ] + [
                        (qc, h) for qc in range(QC) for h in (hpc - 2, hpc - 1)
                    ]
                    for qc, h in norm_order:
                        ql = qc * 512
                        po = (h % 2) * hd
                        hb = h // 2
                        d0 = dip.tile([1, 512], FR, tag="d0")
                        nc.scalar.dma_start(
                            d0, dinv_sb[h:h + 1, ql:ql + 512].bitcast(FR)
                        )
                        bc = pbc.tile([hd, 512], FP, tag="bc")
                        nc.tensor.matmul(
                            bc, lhsT=ones_sb, rhs=d0, start=True, stop=True
                        )
                        nc.vector.tensor_mul(
                            aoT_sb[po:po + hd, hb, ql:ql + 512],
                            aoT_sb[po:po + hd, hb, ql:ql + 512],
                            bc,
                        )
                    for nb in range(NB):
                            ot = osp.tile([128, c], BF, tag="ot")
                            for co in range(c // 512):
                                pt = pso.tile([128, 512], FP, tag="pso")
                                for mb in range(MB):
                                    nc.tensor.matmul(
                                        pt,
                                        lhsT=aoT_sb[:, mb, nb * 128:(nb + 1) * 128],
                                        rhs=wp_sb[:, mb, co * 512:(co + 1) * 512],
                                        start=(mb == 0),
                                        stop=(mb == MB - 1),
                                    )
                                # split PSUM evacuation across ACT and DVE
                                if co == 0:
                                    nc.scalar.copy(
                                        ot[:, co * 512:(co + 1) * 512], pt
                                    )
                                else:
                                    nc.vector.tensor_copy(
                                        ot[:, co * 512:(co + 1) * 512], pt
                                    )
                            nc.sync.dma_start(
                                out.rearrange("(nb p) co -> p nb co", p=128)[:, nb, :],
                                ot,
                            )
    nc.compile()
    return nc


def make_in_maps(x, mask, wq, wk, wv, wp):
    """Host-side sharding: per-core input dict."""
    bf16 = ml_dtypes.bfloat16
    in_maps = []
    for core in range(NCORES):
        b = core // 2
        g = core % 2
        cs = slice(g * 512, (g + 1) * 512)
        in_maps.append({
            "xT": np.ascontiguousarray(x[b].T).astype(bf16),
            "wqT": np.ascontiguousarray(wq[cs, :].T).astype(bf16),
            "wkT": np.ascontiguousarray(wk[cs, :].T).astype(bf16),
            "wvT": np.ascontiguousarray(wv[cs, :].T).astype(bf16),
            "wpT": np.ascontiguousarray(wp[:, cs].T).astype(bf16),
            "maskT": np.ascontiguousarray(mask[b].T).astype(bf16),
        })
    return in_maps


_NC_CACHE = {}


def _get_nc():
    if "nc" not in _NC_CACHE:
        _NC_CACHE["nc"] = build_attention()
    return _NC_CACHE["nc"]


def kernel(x, mask, wq, wk, wv, wp, bp, _trace=False, _trace_kwargs=None):
    x = np.asarray(x, dtype=np.float32)
    mask = np.asarray(mask)
    wq = np.asarray(wq, dtype=np.float32)
    wk = np.asarray(wk, dtype=np.float32)
    wv = np.asarray(wv, dtype=np.float32)
    wp = np.asarray(wp, dtype=np.float32)
    bp = np.asarray(bp, dtype=np.float32)

    nc = _get_nc()
    in_maps = make_in_maps(x, mask, wq, wk, wv, wp)
    kw = {}
    if _trace:
        kw = {"trace": True, **(_trace_kwargs or {})}
    res = run_bass_kernel_spmd(nc, in_maps, list(range(NCORES)), **kw)
    outs = [np.asarray(r["out"], dtype=np.float32) for r in res.results]
    full = np.empty((B, N, DIM), dtype=np.float32)
    for b in range(B):
        full[b] = outs[2 * b] + outs[2 * b + 1] + bp[None, :]
    if _trace:
        return full, res
    return full


if __name__ == "__main__":
    nc = build_attention()
    print("built ok")

